# revision 5
# baseline (speedup 1.0000x reference)
"""Trainium2 Bass kernel for nn_CWDiscriminator (per-class 3-layer MLP).

reference:
    x = inputs.transpose(0, 2, 1)            # (B, C, F)
    h = relu(einsum('bcf,cfg->bcg', x, W1) + b1)
    h = relu(einsum('bcf,cfg->bcg', h, W2) + b2)
    out = einsum('bcf,cf->bc', h, W3) + b3   # (B, C)

B=16384, F=256, C=19. Data-parallel over B across 8 NeuronCores
(B_loc = 2048 per core). Per core, per class c:
  - inputs arrive as (B_loc, F*C) bf16 (host-cast); PE transpose-mode
    converts the f-strided slices into X.T tiles (f on partitions).
  - GEMM1 (bf16): H1.T = W1[c].T @ X.T  -> PSUM, evicted by ACT with
    fused bias+ReLU to fp32r.
  - GEMM2 (fp32r): H2.T = W2[c].T @ H1.T -> PSUM, evicted with
    bias+ReLU to fp32r (ACT/DVE split).
  - GEMM3 (fp32r): lhsT = W3 masked to column c (128, 19); all classes
    accumulate into one shared PSUM (19, b) region, so the final
    eviction is one op per half instead of per class.
Output per core is (C, B_loc) fp32; host transposes and adds b3.
"""

import sys
import types

import numpy as np
import ml_dtypes

B, F, C = 16384, 256, 19
NCORES = 8
B_LOC = B // NCORES          # 2048
SECTIONS = [512, 512, 1024]  # batch columns per PSUM-accum round; small
                             # leading sections so compute starts before the
                             # whole input half has landed in SBUF
assert sum(SECTIONS) == 2048
NCHUNK = 512                 # matmul moving free dim (one fp32 PSUM bank)
FC = F * C                   # 4864

BF16 = ml_dtypes.bfloat16


# ---------------------------------------------------------------------------
# axon environment shims (NTFF profile hook + artifact upload stub) and the
# one-wait-per-instruction legalizer this walrus build requires.
# ---------------------------------------------------------------------------

def _setup_axon_env():
    if 'antenv.axon_hooks' not in sys.modules:
        mod = types.ModuleType('antenv.axon_hooks')
        mod._hook = None
        mod.set_axon_ntff_profile_hook = lambda h: setattr(mod, '_hook', h)
        mod.get_axon_ntff_profile_hook = lambda: mod._hook
        sys.modules['antenv.axon_hooks'] = mod
        try:
            import antenv
            antenv.axon_hooks = mod
        except ImportError:
            pass
        try:
            from trn_agent_boot.trn_boot import _ntff_profile_via_ctypes
            mod._hook = _ntff_profile_via_ctypes('/opt/axon/libaxon_pjrt.so')
        except Exception:
            pass
    import concourse.bass_utils as bu
    bu.upload_artifacts = lambda tmpdir: 'file://' + str(tmpdir)


def _legalize_waits(nc):
    """walrus accepts at most ONE sync wait per engine instruction (2 for
    EventSemaphore). Split extras onto preceding same-engine NoOps."""
    import concourse.mybir as mybir
    n_split = 0
    for fn in nc.m.functions:
        for bb in fn.blocks:
            insts = bb.instructions
            out = []
            for inst in insts:
                si = inst.sync_info
                ow = list(si.on_wait) if si is not None and si.on_wait else []
                cap = 2 if inst.opcode == "EventSemaphore" else 1
                if len(ow) > cap:
                    keep = ow[-cap:]
                    for k, w in enumerate(ow[:-cap]):
                        nop = mybir.InstNoOp(
                            name=f"{inst.name}-wsplit{k}",
                            engine=inst.engine,
                            ins=[],
                            outs=[],
                            sync_info=mybir.SyncInfo(on_wait=[w], on_update=[]),
                        )
                        out.append(nop)
                        n_split += 1
                    inst.sync_info = mybir.SyncInfo(
                        on_wait=keep,
                        on_update=list(si.on_update) if si.on_update else [],
                    )
                out.append(inst)
            insts[:] = out
    return n_split


# ---------------------------------------------------------------------------
# device program
# ---------------------------------------------------------------------------

_CACHE = {}
last_results = None  # BassKernelResults of the most recent run (for test.py)


def _build_program():
    from contextlib import ExitStack
    import concourse.bass as bass
    import concourse.mybir as mybir
    import concourse.tile as tile
    from concourse.masks import make_identity

    F32 = mybir.dt.float32
    F32R = mybir.dt.float32r
    B16 = mybir.dt.bfloat16

    nc = bass.Bass()

    xb = nc.declare_dram_parameter("xb", [B_LOC, FC], B16, isOutput=False)
    w1t = nc.declare_dram_parameter("w1t", [128, C, 2, 2, 128], B16,
                                    isOutput=False)
    w2t = nc.declare_dram_parameter("w2t", [128, C * 2 * 2 * 128], F32,
                                    isOutput=False)
    w3m = nc.declare_dram_parameter("w3m", [128, C * 2 * C], F32,
                                    isOutput=False)
    b1s = nc.declare_dram_parameter("b1s", [128, C, 2], F32, isOutput=False)
    b2s = nc.declare_dram_parameter("b2s", [128, C, 2], F32, isOutput=False)
    out = nc.declare_dram_parameter("out", [C, B_LOC], F32, isOutput=True)

    with ExitStack() as ctx:
        tc = ctx.enter_context(tile.TileContext(nc))

        consts = ctx.enter_context(tc.tile_pool(name="consts", bufs=1))
        wtmp_pool = ctx.enter_context(tc.tile_pool(name="wtmp", bufs=1))
        xraw_pool = ctx.enter_context(tc.tile_pool(name="xraw", bufs=3))
        xt_pool = ctx.enter_context(tc.tile_pool(name="xt", bufs=1))
        h1_pool = ctx.enter_context(tc.tile_pool(name="h1p", bufs=2))
        h2_pool = ctx.enter_context(tc.tile_pool(name="h2p", bufs=2))
        out_pool = ctx.enter_context(tc.tile_pool(name="outp", bufs=1))

        ps_g = ctx.enter_context(
            tc.tile_pool(name="ps_g", bufs=3, space="PSUM"))
        ps_3 = ctx.enter_context(
            tc.tile_pool(name="ps_3", bufs=1, space="PSUM"))

        # ---- one-time constants -----------------------------------------
        ident = consts.tile([128, 128], B16)
        make_identity(nc, ident[:])

        w1sb = consts.tile([128, C, 2, 2, 128], B16)
        nc.sync.dma_start(w1sb[:], w1t[:])

        # W2/W3 must be *rounded to fp32r* by a compute op before fp32r
        # matmuls may consume them; DMA as fp32 then convert on DVE.
        w2sb = consts.tile([128, C * 2 * 2 * 128], F32R)
        w3sb = consts.tile([128, C * 2 * C], F32R)
        NW2 = C * 2 * 2 * 128  # 9728
        for i in range(4):
            wtmp = xraw_pool.tile([128, NW2 // 4], F32, tag="xr")
            nc.sync.dma_start(wtmp[:], w2t[:, i * (NW2 // 4):(i + 1) * (NW2 // 4)])
            nc.vector.tensor_copy(
                w2sb[:, i * (NW2 // 4):(i + 1) * (NW2 // 4)], wtmp[:])
        w3tmp = wtmp_pool.tile([128, C * 2 * C], F32, tag="w3tmp")
        nc.sync.dma_start(w3tmp[:], w3m[:])
        nc.vector.tensor_copy(w3sb[:], w3tmp[:])

        b1sb = consts.tile([128, C, 2], F32)
        nc.sync.dma_start(b1sb[:], b1s[:])
        b2sb = consts.tile([128, C, 2], F32)
        nc.sync.dma_start(b2sb[:], b2s[:])

        out_sb = out_pool.tile([C, B_LOC], F32)

        w1v = w1sb[:]
        w2v = w2sb[:].rearrange("p (c k m j) -> p c k m j", c=C, k=2, m=2)
        w3v = w3sb[:].rearrange("p (c k q) -> p c k q", c=C, k=2)

        CK = C * 2  # 38 (c,k) pairs per gb-tile of transposes

        sec_start = 0
        for h, HALF in enumerate(SECTIONS):
            GB = HALF // 128
            # ---- load the section's row-blocks -------------------------
            xraws = []
            for gb in range(GB):
                xr = xraw_pool.tile([128, FC], B16, tag="xr")
                r0 = sec_start + gb * 128
                nc.sync.dma_start(xr[:], xb[r0:r0 + 128, :])
                xraws.append(xr)

            # X.T slab for this section: [p=f_low, (c,k) flattened, b]
            xt = xt_pool.tile([128, CK, HALF], B16, tag="xt")
            xtv = xt[:]

            # ---- transposes: per row-block, per pack of 8 (c,k) --------
            for gb in range(GB):
                xrv = xraws[gb][:].rearrange("p (f c) -> p f c", c=C)
                for p0 in range(0, CK, 8):
                    npk = min(8, CK - p0)
                    pk = ps_g.tile([128, 1024], B16, tag="pg")  # slot-shared
                    for j in range(npk):
                        ck = p0 + j
                        c, k = ck // 2, ck % 2
                        src = xrv[:, k * 128:(k + 1) * 128, c]
                        nc.tensor.transpose(
                            pk[:, j * 128:(j + 1) * 128], src, ident[:])
                    dst = xtv[:, p0:p0 + npk, gb * 128:(gb + 1) * 128]
                    nc.vector.tensor_copy(dst, pk[:, :npk * 128].rearrange(
                        "p (j q) -> p j q", j=npk))

            # ---- per-class MLP, software-pipelined over classes --------
            # iteration cc runs GEMM1(cc), GEMM2(cc-1), GEMM3(cc-2) so PE
            # never waits on a PSUM eviction of its own output.
            ps3 = ps_3.tile([C, HALF], mybir.dt.float32, tag="ps3")
            h1_t = [None, None]
            h2_t = [None, None]
            for cc in range(C + 2):
                if cc < C:
                    c = cc
                    h1 = h1_pool.tile([128, 2, HALF], F32R, tag="h1")
                    h1_t[c % 2] = h1
                    for m in range(2):
                        pg = ps_g.tile([128, HALF], mybir.dt.float32,
                                       tag="pg")
                        for k in range(2):
                            for n2 in range(HALF // NCHUNK):
                                nc.tensor.matmul(
                                    pg[:, n2 * NCHUNK:(n2 + 1) * NCHUNK],
                                    w1v[:, c, k, m, :],
                                    xtv[:, c * 2 + k,
                                        n2 * NCHUNK:(n2 + 1) * NCHUNK],
                                    start=(k == 0), stop=(k == 1))
                        nc.scalar.activation(
                            h1[:, m, :], pg[:],
                            mybir.ActivationFunctionType.Relu,
                            bias=b1sb[:, c, m:m+1])
                if 1 <= cc <= C:
                    c = cc - 1
                    h1 = h1_t[c % 2]
                    h2 = h2_pool.tile([128, 2, HALF], F32R, tag="h2")
                    h2_t[c % 2] = h2
                    for m in range(2):
                        pg = ps_g.tile([128, HALF], mybir.dt.float32,
                                       tag="pg")
                        for k in range(2):
                            for n2 in range(HALF // NCHUNK):
                                nc.tensor.matmul(
                                    pg[:, n2 * NCHUNK:(n2 + 1) * NCHUNK],
                                    w2v[:, c, k, m, :],
                                    h1[:, k, n2 * NCHUNK:(n2 + 1) * NCHUNK],
                                    start=(k == 0), stop=(k == 1))
                        if m == 0:
                            nc.scalar.activation(
                                h2[:, m, :], pg[:],
                                mybir.ActivationFunctionType.Relu,
                                bias=b2sb[:, c, m:m+1])
                        else:
                            nc.vector.tensor_scalar(
                                h2[:, m, :], pg[:],
                                b2sb[:, c, m:m+1], 0.0,
                                mybir.AluOpType.add, mybir.AluOpType.max)
                if cc >= 2:
                    c = cc - 2
                    h2 = h2_t[c % 2]
                    for k in range(2):
                        for n2 in range(HALF // NCHUNK):
                            nc.tensor.matmul(
                                ps3[:, n2 * NCHUNK:(n2 + 1) * NCHUNK],
                                w3v[:, c, k, :],
                                h2[:, k, n2 * NCHUNK:(n2 + 1) * NCHUNK],
                                start=(c == 0 and k == 0),
                                stop=(c == C - 1 and k == 1))

            nc.vector.tensor_copy(
                out_sb[:, sec_start:sec_start + HALF], ps3[:])
            sec_start += HALF

        nc.sync.dma_start(out[:], out_sb[:])

    _legalize_waits(nc)
    return nc


def _get_program():
    if 'nc' not in _CACHE:
        _setup_axon_env()
        _CACHE['nc'] = _build_program()
    return _CACHE['nc']


# ---------------------------------------------------------------------------
# host wrapper
# ---------------------------------------------------------------------------

def kernel(inputs, W1, b1, W2, b2, W3, b3):
    global last_results
    from concourse.bass_utils import run_bass_kernel_spmd

    nc = _get_program()

    inputs = np.asarray(inputs)
    W1 = np.asarray(W1, dtype=np.float32)
    b1 = np.asarray(b1, dtype=np.float32)
    W2 = np.asarray(W2, dtype=np.float32)
    b2 = np.asarray(b2, dtype=np.float32)
    W3 = np.asarray(W3, dtype=np.float32)
    b3 = np.asarray(b3, dtype=np.float32)

    xbf = np.ascontiguousarray(inputs.reshape(B, FC)).astype(BF16)

    # lhsT tiles: w1t[p, c, k, m, j] = W1[c, 128k+p, 128m+j]
    w1t = np.ascontiguousarray(
        W1.reshape(C, 2, 128, 2, 128).transpose(2, 0, 1, 3, 4)).astype(BF16)
    w2t = np.ascontiguousarray(
        W2.reshape(C, 2, 128, 2, 128).transpose(2, 0, 1, 3, 4)
    ).reshape(128, C * 2 * 2 * 128).astype(np.float32)
    # w3m[p, c, k, c'] = (c'==c) * W3[c, 128k+p]
    w3m = np.zeros((128, C, 2, C), dtype=np.float32)
    for c in range(C):
        w3m[:, c, 0, c] = W3[c, :128]
        w3m[:, c, 1, c] = W3[c, 128:]
    w3m = w3m.reshape(128, C * 2 * C)
    # b1s[p, c, m] = b1[c, 128m+p]
    b1s = np.ascontiguousarray(
        b1.reshape(C, 2, 128).transpose(2, 0, 1)).astype(np.float32)
    b2s = np.ascontiguousarray(
        b2.reshape(C, 2, 128).transpose(2, 0, 1)).astype(np.float32)

    core_ids = list(range(NCORES))
    in_maps = []
    for i in core_ids:
        in_maps.append({
            "xb": xbf[i * B_LOC:(i + 1) * B_LOC],
            "w1t": w1t, "w2t": w2t, "w3m": w3m, "b1s": b1s, "b2s": b2s,
        })

    import os
    trace = bool(os.environ.get("BASS_TRACE"))
    res = run_bass_kernel_spmd(nc, in_maps, core_ids, trace=trace)
    last_results = res

    out_full = np.empty((B, C), dtype=np.float32)
    for i in core_ids:
        out_full[i * B_LOC:(i + 1) * B_LOC] = res.results[i]["out"].T
    out_full += b3[None, :]
    return out_full
